# revision 14
# baseline (speedup 1.0000x reference)
"""CGCConv-style GNN message passing kernel for 8 Trainium2 NeuronCores.

Reference computation (per edge e: src j -> dst i):
    msgs = edge_weight[:, None] * x[src] * pagerank[src][:, None]      # [E, D]
    aggr = segment_sum(msgs, dst, N)                                    # [N, D]
    out  = (aggr + x) @ W.T + b                                         # [N, D]

Strategy (dst-sharded, host-expanded dense message stream; no collectives):
  - dst nodes are assigned to cores by balanced degree (LPT), then within a
    core to 784 octant-bins (window w in 0..48, section s in 0..1, octant A
    in 0..7) of exactly 8 dst positions each, LPT-balancing the bin edge
    counts toward <= 128.
  - Each octant-bin owns exactly one 128-slot tile; every in-bin edge gets a
    slot (partition). Host writes xexp[slot] = x[src_e] (fp16) so the device
    reads ONE dense sequential stream instead of per-edge gathers.
  - Per tile, the dst octant A is static, so the aggregation matmul is
    8-wide: ps[:, s*64+A*8 : +8] += G_tile^T @ OH8_tile where
    OH8[p, b] = w_e*pr_e * onehot8(pos_e % 8). OH8 is built on DVE from two
    per-slot tables (wpr, drB) with one is_equal + one mult per call.
  - Bin overflow edges (few hundred per core) go to per-call spill tiles
    with classic 64-wide one-hot vcols (drA/drB outer product).
  - Update: ps starts from x (identity matmul); final linear per window is
    one matmul with lhsT=[aggr.T; ones] ([97, 128]) and rhs=[W.T; b].
"""

import sys

for _p in ("/opt/trn_rl_repo",):
    if _p not in sys.path:
        sys.path.insert(0, _p)

import numpy as np

import concourse.mybir as mybir
import concourse.tile as tile
from concourse import bacc
from concourse.bass_utils import run_bass_kernel_spmd
from concourse.masks import make_identity

F32 = mybir.dt.float32
F16 = mybir.dt.float16

N_NODES = 50000
D = 96
NCORES = 8
WIN = 128
NW = 49
PER = WIN * NW       # 6272 dst nodes per core
NPAD = PER * NCORES  # 50176
GW = 7               # windows per group/call
NG = 7               # groups
NBIN_W = 16          # (s, A) bins per window
NBINS = NW * NBIN_W  # 784 octant-bins per core
TILES_MAIN = GW * NBIN_W  # 112 main tiles per call

_LAST = {}


def _lpt_assign(loads, nitems_per_bin, nbins, order):
    """Greedy LPT: assign items (in given order) to the min-loaded bin with
    space. loads: per-item weights. Returns bin index per item."""
    import heapq

    heap = [(0.0, b) for b in range(nbins)]
    heapq.heapify(heap)
    fill = np.zeros(nbins, np.int64)
    out = np.zeros(len(loads), np.int64)
    stash = []
    for it in order:
        while True:
            load, b = heapq.heappop(heap)
            if fill[b] < nitems_per_bin:
                break
            stash.append((load, b))
        out[it] = b
        fill[b] += 1
        heapq.heappush(heap, (load + loads[it], b))
        for ent in stash:
            heapq.heappush(heap, ent)
        stash.clear()
    return out


def _host_prep(x, edge_index, edge_weight, pagerank):
    src = np.asarray(edge_index[0], dtype=np.int64)
    dst = np.asarray(edge_index[1], dtype=np.int64)
    ew = np.asarray(edge_weight, dtype=np.float32)
    pr = np.asarray(pagerank, np.float32)
    E = len(src)

    # --- dst -> core assignment, balanced by degree (LPT over nodes) ---
    deg_all = np.bincount(dst, minlength=NPAD).astype(np.int64)
    order = np.argsort(-deg_all, kind="stable")
    node_core = _lpt_assign(deg_all.astype(np.float64), PER, NCORES, order)
    core = node_core[dst]

    # --- per core: nodes -> octant-bins (8 nodes per bin), LPT on degree ---
    node_bin = np.zeros(NPAD, np.int64)   # bin in [0, 784)
    node_pos8 = np.zeros(NPAD, np.int64)  # position within bin [0, 8)
    for c in range(NCORES):
        nodes = np.where(node_core == c)[0]
        dg = deg_all[nodes].astype(np.float64)
        order_c = np.argsort(-dg, kind="stable")
        b = _lpt_assign(dg, 8, NBINS, order_c)
        node_bin[nodes] = b
        # position within bin: assign by arrival order
        posc = np.zeros(NBINS, np.int64)
        p8 = np.zeros(len(nodes), np.int64)
        for it in order_c:
            p8[it] = posc[b[it]]
            posc[b[it]] += 1
        node_pos8[nodes] = p8[np.arange(len(nodes))]

    # decode bin -> (w, s, A); dst position within window
    node_w = node_bin // NBIN_W
    node_s = (node_bin % NBIN_W) // 8
    node_A = node_bin % 8
    node_pos = node_s * 64 + node_A * 8 + node_pos8  # [0, 128)

    # --- edge -> slot assignment ---
    e_bin = node_bin[dst]            # [E]
    e_w = node_w[dst]
    e_s = node_s[dst]
    e_A = node_A[dst]
    e_g = e_w // GW
    e_drb = node_pos8[dst]           # pos % 8 within octant

    # rank within (core, bin)
    key = core * NBINS + e_bin
    order_e = np.argsort(key, kind="stable")
    ko = key[order_e]
    starts = np.searchsorted(ko, np.arange(NCORES * NBINS))
    rank = np.empty(E, np.int64)
    rank[order_e] = np.arange(E) - starts[ko]

    main = rank < WIN
    spill = ~main

    # main slot: tile j (static per bin within call), partition p = rank
    # call layout: [main tiles (112) | spill tiles (SP)] per call
    bin_tile_in_call = (e_w % GW) * NBIN_W + e_s * 8 + e_A  # [0,112)

    # --- spill layout: per (core, g): sections (w,s) runs padded to caps ---
    sp_counts = np.zeros((NCORES, NW, 2), np.int64)
    np.add.at(sp_counts, (core[spill], e_w[spill], e_s[spill]), 1)
    cap_sp = sp_counts.max(axis=0)  # [NW, 2]
    # spill run base per (w, s), within call spill region
    sp_base = np.zeros((NW, 2), np.int64)
    sp_tiles = np.zeros(NG, np.int64)
    for g in range(NG):
        off = 0
        for w in range(g * GW, (g + 1) * GW):
            for s in range(2):
                sp_base[w, s] = off
                off += int(cap_sp[w, s])
        sp_tiles[g] = (off + WIN - 1) // WIN
    SP = int(sp_tiles.max())
    M_CALL = TILES_MAIN + SP
    M_TOT = NG * M_CALL

    # spill vcols: per g: (tile, w, s) for each spill-tile overlapping run
    sp_vcols = [[] for _ in range(NG)]  # list of (j_in_call, w, s)
    sp_vcol_id = {}
    for g in range(NG):
        for w in range(g * GW, (g + 1) * GW):
            for s in range(2):
                a = int(sp_base[w, s])
                b_ = a + int(cap_sp[w, s])
                if b_ <= a:
                    continue
                for j in range(a // WIN, (b_ - 1) // WIN + 1):
                    sp_vcol_id[(g, j, w, s)] = len(sp_vcols[g])
                    sp_vcols[g].append((TILES_MAIN + j, w, s))
    NVSP = max(len(v) for v in sp_vcols) if any(sp_vcols) else 0
    NVSP_TOT = NG * max(NVSP, 1)

    # spill slot: rank within (core, w, s) among spill edges
    skey = (core * NW + e_w) * 2 + e_s
    so = np.argsort(skey[spill], kind="stable")
    sko = skey[spill][so]
    sstarts = np.searchsorted(sko, np.arange(NCORES * NW * 2))
    srank = np.empty(spill.sum(), np.int64)
    srank[so] = np.arange(spill.sum()) - sstarts[sko]

    # --- build per-core upload arrays ---
    x16 = np.zeros((NPAD, D), np.float16)
    x16[:N_NODES] = np.asarray(x, np.float32).astype(np.float16)
    ew16 = ew.astype(np.float16)
    wpr = (ew * pr[src]).astype(np.float16)

    xexp = np.zeros((NCORES, WIN, M_TOT, D), np.float16)
    wpr_t = np.zeros((NCORES, WIN, NG * TILES_MAIN), np.float16)
    drb_t = np.full((NCORES, WIN, NG * TILES_MAIN), -1.0, np.float16)
    wpr_sp = np.zeros((NCORES, WIN, NVSP_TOT), np.float16)
    dra_sp = np.full((NCORES, WIN, NVSP_TOT), -1.0, np.float16)
    drb_sp = np.full((NCORES, WIN, NVSP_TOT), -1.0, np.float16)

    # main edges
    em = main
    j_glob = e_g[em] * M_CALL + bin_tile_in_call[em]
    jm_glob = e_g[em] * TILES_MAIN + bin_tile_in_call[em]
    p_m = rank[em]
    xexp[core[em], p_m, j_glob] = x16[src[em]]
    wpr_t[core[em], p_m, jm_glob] = wpr[em]
    drb_t[core[em], p_m, jm_glob] = e_drb[em].astype(np.float16)

    # spill edges
    es_idx = np.where(spill)[0]
    sw, ss, sg, sc = e_w[es_idx], e_s[es_idx], e_g[es_idx], core[es_idx]
    soff = sp_base[sw, ss] + srank
    sj = soff // WIN          # spill tile within call spill region
    sp_p = soff % WIN
    vids = np.array([sp_vcol_id[(g_, j_, w_, s_)]
                     for g_, j_, w_, s_ in zip(sg, sj, sw, ss)], np.int64) \
        if len(es_idx) else np.zeros(0, np.int64)
    if len(es_idx):
        v_glob = sg * max(NVSP, 1) + vids
        j_sp_glob = sg * M_CALL + TILES_MAIN + sj
        xexp[sc, sp_p, j_sp_glob] = x16[src[es_idx]]
        wpr_sp[sc, sp_p, v_glob] = wpr[es_idx]
        pos_sp = node_pos[dst[es_idx]]
        dra_sp[sc, sp_p, v_glob] = ((pos_sp % 64) // 8).astype(np.float16)
        drb_sp[sc, sp_p, v_glob] = (pos_sp % 8).astype(np.float16)

    # xw: dense x rows per (pos, w) for the +x residual
    rows = np.zeros((NCORES, WIN, NW), np.int64)
    for c in range(NCORES):
        nodes = np.where(node_core == c)[0]
        rows[c, node_pos[nodes], node_w[nodes]] = nodes
    xw = x16[rows]  # [NCORES, 128, NW, D]

    return dict(M_CALL=M_CALL, M_TOT=M_TOT, SP=SP, NVSP=max(NVSP, 1),
                NVSP_TOT=NVSP_TOT, sp_vcols=sp_vcols, rows=rows,
                xexp=xexp, wpr_t=wpr_t, drb_t=drb_t, wpr_sp=wpr_sp,
                dra_sp=dra_sp, drb_sp=drb_sp, xw=xw,
                spill_count=int(spill.sum()))


def _build_nc(prep):
    M_CALL, SP = prep["M_CALL"], prep["SP"]
    NVSP = prep["NVSP"]
    sp_vcols = prep["sp_vcols"]

    NTM = NG * TILES_MAIN
    NVT = prep["NVSP_TOT"]

    nc = bacc.Bacc(num_devices=NCORES)
    xexp_t = nc.dram_tensor("xexp", [WIN, prep["M_TOT"] * D], F16,
                            kind="ExternalInput")
    mtab_t = nc.dram_tensor("mtab", [WIN, 2 * NTM], F16, kind="ExternalInput")
    stab_t = nc.dram_tensor("stab", [WIN, 3 * NVT], F16, kind="ExternalInput")
    xwb_t = nc.dram_tensor("xwb", [WIN, NW * D + D], F16, kind="ExternalInput")
    out_t = nc.dram_tensor("out", [WIN, NW, D], F16, kind="ExternalOutput")

    with tile.TileContext(nc) as tc:
        from contextlib import ExitStack

        with ExitStack() as ctx:
            const = ctx.enter_context(tc.tile_pool(name="const", bufs=1))
            gp = ctx.enter_context(tc.tile_pool(name="gp", bufs=2))
            ohp = ctx.enter_context(tc.tile_pool(name="ohp", bufs=2))
            osp = ctx.enter_context(tc.tile_pool(name="osp", bufs=2))
            abp = ctx.enter_context(tc.tile_pool(name="abp", bufs=2))
            aggp = ctx.enter_context(tc.tile_pool(name="aggp", bufs=3))
            psw = ctx.enter_context(tc.tile_pool(name="psw", bufs=1, space="PSUM"))
            psr = ctx.enter_context(tc.tile_pool(name="psr", bufs=1, space="PSUM"))

            # G for call 0 first: it is the longest transfer on the
            # critical path, so it must hit the DMA engines before the
            # constant tables.
            G0 = gp.tile([WIN, M_CALL, D], F16, tag="g0")
            nc.sync.dma_start(out=G0[:, :, :], in_=xexp_t[:, :M_CALL * D])

            mtab = const.tile([WIN, 2 * NTM], F16)
            nc.sync.dma_start(out=mtab[:, :], in_=mtab_t[:, :])
            wprm = mtab[:, :NTM]
            drbm = mtab[:, NTM:]
            xwb = const.tile([WIN, NW * D + D], F16)
            nc.sync.dma_start(out=xwb[:, :], in_=xwb_t[:, :])
            wbt = xwb[:D + 1, NW * D:]
            stab = const.tile([WIN, 3 * NVT], F16)
            nc.sync.dma_start(out=stab[:, :], in_=stab_t[:, :])
            wprs = stab[:, :NVT]
            dras = stab[:, NVT:2 * NVT]
            drbs = stab[:, 2 * NVT:]

            ident16 = const.tile([WIN, WIN], F16)
            make_identity(nc, ident16[:, :])
            iota8 = const.tile([WIN, 8], F16)
            nc.gpsimd.iota(iota8[:, :], pattern=[[1, 8]], base=0,
                           channel_multiplier=0,
                           allow_small_or_imprecise_dtypes=True)

            outr = const.tile([WIN, NW, D], F16)

            # pre-set the ones row of the agg buffers (one per window-in-group
            # so a whole group's updates can be in flight at once)
            aggs = []
            for k in range(GW):
                agg = aggp.tile([D + 1, WIN], F16, tag=f"agg{k}")
                nc.vector.memset(agg[D:D + 1, :], 1.0)
                aggs.append(agg)

            for g in range(NG):
                if g == 0:
                    G = G0
                else:
                    G = gp.tile([WIN, M_CALL, D], F16, tag=f"g{g % 2}")
                    nc.sync.dma_start(
                        out=G[:, :, :],
                        in_=xexp_t[:, g * M_CALL * D:(g + 1) * M_CALL * D])

                # 8-wide one-hot for the 112 main tiles of this call
                OH8 = ohp.tile([WIN, TILES_MAIN, 8], F16, tag=f"oh{g % 2}")
                nc.vector.tensor_tensor(
                    out=OH8[:, :, :],
                    in0=iota8[:, None, :].to_broadcast([WIN, TILES_MAIN, 8]),
                    in1=drbm[:, g * TILES_MAIN:(g + 1) * TILES_MAIN, None]
                        .to_broadcast([WIN, TILES_MAIN, 8]),
                    op=mybir.AluOpType.is_equal,
                )
                nc.vector.tensor_tensor(
                    out=OH8[:, :, :],
                    in0=OH8[:, :, :],
                    in1=wprm[:, g * TILES_MAIN:(g + 1) * TILES_MAIN, None]
                        .to_broadcast([WIN, TILES_MAIN, 8]),
                    op=mybir.AluOpType.mult,
                )

                # 64-wide one-hot for spill vcols of this call
                nv = len(sp_vcols[g])
                OHS = None
                if nv:
                    v0 = g * NVSP
                    ohA = abp.tile([WIN, NVSP, 8], F16, tag=f"a{g % 2}")
                    nc.vector.tensor_tensor(
                        out=ohA[:, :nv, :],
                        in0=iota8[:, None, :].to_broadcast([WIN, nv, 8]),
                        in1=dras[:, v0:v0 + nv, None].to_broadcast([WIN, nv, 8]),
                        op=mybir.AluOpType.is_equal,
                    )
                    nc.vector.tensor_tensor(
                        out=ohA[:, :nv, :],
                        in0=ohA[:, :nv, :],
                        in1=wprs[:, v0:v0 + nv, None].to_broadcast([WIN, nv, 8]),
                        op=mybir.AluOpType.mult,
                    )
                    ohB = abp.tile([WIN, NVSP, 8], F16, tag=f"b{g % 2}")
                    nc.vector.tensor_tensor(
                        out=ohB[:, :nv, :],
                        in0=iota8[:, None, :].to_broadcast([WIN, nv, 8]),
                        in1=drbs[:, v0:v0 + nv, None].to_broadcast([WIN, nv, 8]),
                        op=mybir.AluOpType.is_equal,
                    )
                    OHS = osp.tile([WIN, NVSP, 8, 8], F16, tag=f"o{g % 2}")
                    nc.vector.tensor_tensor(
                        out=OHS[:, :nv, :, :],
                        in0=ohA[:, :nv, :, None].to_broadcast([WIN, nv, 8, 8]),
                        in1=ohB[:, :nv, None, :].to_broadcast([WIN, nv, 8, 8]),
                        op=mybir.AluOpType.mult,
                    )

                pss = []
                for wl in range(GW):
                    w = g * GW + wl
                    myspill = [(k, j, s) for k, (j, w_, s) in enumerate(sp_vcols[g])
                               if w_ == w]
                    ntot = NBIN_W + len(myspill)
                    ps = psw.tile([D, WIN], F32, tag=f"ps{wl % 6}")
                    nc.tensor.matmul(out=ps[:, :], lhsT=xwb[:, w * D:(w + 1) * D],
                                     rhs=ident16[:, :], start=True, stop=False,
                                     skip_group_check=True)
                    done = 0
                    for s in range(2):
                        for A in range(8):
                            jm = wl * NBIN_W + s * 8 + A
                            done += 1
                            nc.tensor.matmul(
                                out=ps[:, s * 64 + A * 8: s * 64 + A * 8 + 8],
                                lhsT=G[:, jm, :],
                                rhs=OH8[:, jm, :],
                                start=False, stop=(done == ntot),
                                skip_group_check=True,
                            )
                    for (k, j, s) in myspill:
                        done += 1
                        nc.tensor.matmul(
                            out=ps[:, s * 64:(s + 1) * 64],
                            lhsT=G[:, j, :],
                            rhs=OHS[:, k, :, :],
                            start=False, stop=(done == ntot),
                            skip_group_check=True,
                        )
                    pss.append(ps)
                    nc.scalar.copy(out=aggs[wl][:D, :], in_=ps[:, :])
                for wl in range(GW):
                    w = g * GW + wl
                    rp = psr.tile([WIN, D], F32, tag=f"rp{wl % 2}")
                    nc.tensor.matmul(out=rp[:, :], lhsT=aggs[wl][:, :],
                                     rhs=wbt[:, :], start=True, stop=True,
                                     skip_group_check=True)
                    nc.vector.tensor_copy(out=outr[:, w, :], in_=rp[:, :])
                nc.sync.dma_start(
                    out=out_t[:, g * GW:(g + 1) * GW, :],
                    in_=outr[:, g * GW:(g + 1) * GW, :])

    nc.compile()
    return nc


def kernel(x, edge_index, edge_weight, pagerank, W, b):
    x = np.asarray(x, np.float32)
    pr = np.asarray(pagerank, np.float32)
    W = np.asarray(W, np.float32)
    b = np.asarray(b, np.float32)

    prep = _host_prep(x, edge_index, edge_weight, pr)
    nc = _build_nc(prep)

    wbt = np.zeros((WIN, D), np.float16)
    wbt[:D] = W.T.astype(np.float16)
    wbt[D] = b.astype(np.float16)

    in_maps = []
    for c in range(NCORES):
        mtab = np.concatenate([prep["wpr_t"][c], prep["drb_t"][c]], axis=1)
        stab = np.concatenate([prep["wpr_sp"][c], prep["dra_sp"][c],
                               prep["drb_sp"][c]], axis=1)
        xwb = np.concatenate([prep["xw"][c].reshape(WIN, NW * D), wbt], axis=1)
        in_maps.append({
            "xexp": prep["xexp"][c].reshape(WIN, prep["M_TOT"] * D),
            "mtab": np.ascontiguousarray(mtab),
            "stab": np.ascontiguousarray(stab),
            "xwb": np.ascontiguousarray(xwb),
        })

    import time

    t0 = time.time()
    res = run_bass_kernel_spmd(nc, in_maps, core_ids=list(range(NCORES)))
    _LAST.update(nc=nc, run_wall_s=time.time() - t0)

    rows = prep["rows"]
    out = np.zeros((NPAD, D), np.float32)
    for c in range(NCORES):
        o = res.results[c]["out"].astype(np.float32)  # [128, NW, 96]
        out[rows[c]] = o
    return out[:N_NODES]


# revision 27
# speedup vs baseline: 1.1476x; 1.1476x over previous
"""CGCConv-style GNN message passing kernel for 8 Trainium2 NeuronCores.

Reference computation (per edge e: src j -> dst i):
    msgs = edge_weight[:, None] * x[src] * pagerank[src][:, None]      # [E, D]
    aggr = segment_sum(msgs, dst, N)                                    # [N, D]
    out  = (aggr + x) @ W.T + b                                         # [N, D]

Strategy (dst-sharded, host-expanded dense message stream; no collectives):
  - dst nodes are assigned to cores by balanced degree (LPT), then within a
    core to 784 octant-bins (window w in 0..48, section s in 0..1, octant A
    in 0..7) of exactly 8 dst positions each, LPT-balancing the bin edge
    counts toward <= 128.
  - Each octant-bin owns exactly one 128-slot tile; every in-bin edge gets a
    slot (partition). Host writes xexp[slot] = x[src_e] (fp16) so the device
    reads ONE dense sequential stream instead of per-edge gathers.
  - Per tile, the dst octant A is static, so the aggregation matmul is
    8-wide: ps[:, s*64+A*8 : +8] += G_tile^T @ OH8_tile where
    OH8[p, b] = w_e*pr_e * onehot8(pos_e % 8). OH8 is built on DVE from two
    per-slot tables (wpr, drB) with one is_equal + one mult per call.
  - Bin overflow edges (few hundred per core) go to per-call spill tiles
    with classic 64-wide one-hot vcols (drA/drB outer product).
  - Update: ps starts from x (identity matmul); final linear per window is
    one matmul with lhsT=[aggr.T; ones] ([97, 128]) and rhs=[W.T; b].
"""

import sys

for _p in ("/opt/trn_rl_repo",):
    if _p not in sys.path:
        sys.path.insert(0, _p)

import ml_dtypes
import numpy as np

import concourse.mybir as mybir
import concourse.tile as tile
from concourse import bacc
from concourse.bass_utils import run_bass_kernel_spmd
from concourse.masks import make_identity

F32 = mybir.dt.float32
F16 = mybir.dt.float16
F8 = mybir.dt.float8e4
NP_F8 = ml_dtypes.float8_e4m3
TAU = 0.4  # edges with w*pr above this get an fp16 residual correction

N_NODES = 50000
D = 96
NCORES = 8
WIN = 128
NW = 49
PER = WIN * NW       # 6272 dst nodes per core
NPAD = PER * NCORES  # 50176
GW = 7               # windows per group/call
NG = 7               # groups
NBIN_W = 16          # (s, A) bins per window
NBINS = NW * NBIN_W  # 784 octant-bins per core
TILES_MAIN = GW * NBIN_W  # 112 main tiles per call

_LAST = {}


def _lpt_assign(loads, nitems_per_bin, nbins, order):
    """Greedy LPT: assign items (in given order) to the min-loaded bin with
    space. loads: per-item weights. Returns bin index per item."""
    import heapq

    heap = [(0.0, b) for b in range(nbins)]
    heapq.heapify(heap)
    fill = np.zeros(nbins, np.int64)
    out = np.zeros(len(loads), np.int64)
    stash = []
    for it in order:
        while True:
            load, b = heapq.heappop(heap)
            if fill[b] < nitems_per_bin:
                break
            stash.append((load, b))
        out[it] = b
        fill[b] += 1
        heapq.heappush(heap, (load + loads[it], b))
        for ent in stash:
            heapq.heappush(heap, ent)
        stash.clear()
    return out


def _host_prep(x, edge_index, edge_weight, pagerank):
    src = np.asarray(edge_index[0], dtype=np.int64)
    dst = np.asarray(edge_index[1], dtype=np.int64)
    ew = np.asarray(edge_weight, dtype=np.float32)
    pr = np.asarray(pagerank, np.float32)
    E = len(src)

    # --- dst -> core assignment, balanced by degree (LPT over nodes) ---
    deg_all = np.bincount(dst, minlength=NPAD).astype(np.int64)
    order = np.argsort(-deg_all, kind="stable")
    node_core = _lpt_assign(deg_all.astype(np.float64), PER, NCORES, order)
    core = node_core[dst]

    # --- per core: nodes -> octant-bins (8 nodes per bin), LPT on degree ---
    node_bin = np.zeros(NPAD, np.int64)   # bin in [0, 784)
    node_pos8 = np.zeros(NPAD, np.int64)  # position within bin [0, 8)
    for c in range(NCORES):
        nodes = np.where(node_core == c)[0]
        dg = deg_all[nodes].astype(np.float64)
        order_c = np.argsort(-dg, kind="stable")
        b = _lpt_assign(dg, 8, NBINS, order_c)
        node_bin[nodes] = b
        # position within bin: assign by arrival order
        posc = np.zeros(NBINS, np.int64)
        p8 = np.zeros(len(nodes), np.int64)
        for it in order_c:
            p8[it] = posc[b[it]]
            posc[b[it]] += 1
        node_pos8[nodes] = p8[np.arange(len(nodes))]

    # decode bin -> (w, s, A); dst position within window
    node_w = node_bin // NBIN_W
    node_s = (node_bin % NBIN_W) // 8
    node_A = node_bin % 8
    node_pos = node_s * 64 + node_A * 8 + node_pos8  # [0, 128)

    # --- edge -> slot assignment ---
    e_bin = node_bin[dst]            # [E]
    e_w = node_w[dst]
    e_s = node_s[dst]
    e_A = node_A[dst]
    e_g = e_w // GW
    e_drb = node_pos8[dst]           # pos % 8 within octant

    # rank within (core, bin)
    key = core * NBINS + e_bin
    order_e = np.argsort(key, kind="stable")
    ko = key[order_e]
    starts = np.searchsorted(ko, np.arange(NCORES * NBINS))
    rank = np.empty(E, np.int64)
    rank[order_e] = np.arange(E) - starts[ko]

    main = rank < WIN
    spill = ~main

    # main slot: tile j (static per bin within call), partition p = rank
    bin_tile_in_call = (e_w % GW) * NBIN_W + e_s * 8 + e_A  # [0,112)

    # --- aux section: bin-overflow edges (full fp16 x) plus fp8-residual
    # corrections for high-weight main edges ---
    wpr_f = ew * pr[src]
    aux = spill | (main & (wpr_f > TAU))
    sp_counts = np.zeros((NCORES, NW, 2), np.int64)
    np.add.at(sp_counts, (core[aux], e_w[aux], e_s[aux]), 1)
    cap_sp = sp_counts.max(axis=0)  # [NW, 2]
    # spill run base per (w, s), within call spill region
    sp_base = np.zeros((NW, 2), np.int64)
    sp_tiles = np.zeros(NG, np.int64)
    for g in range(NG):
        off = 0
        for w in range(g * GW, (g + 1) * GW):
            for s in range(2):
                sp_base[w, s] = off
                off += int(cap_sp[w, s])
        sp_tiles[g] = (off + WIN - 1) // WIN
    SPA = int(sp_tiles.max())

    # aux vcols: per g: (tile, w, s) for each aux-tile overlapping a run
    sp_vcols = [[] for _ in range(NG)]  # list of (j_aux, w, s)
    sp_vcol_id = {}
    for g in range(NG):
        for w in range(g * GW, (g + 1) * GW):
            for s in range(2):
                a = int(sp_base[w, s])
                b_ = a + int(cap_sp[w, s])
                if b_ <= a:
                    continue
                for j in range(a // WIN, (b_ - 1) // WIN + 1):
                    sp_vcol_id[(g, j, w, s)] = len(sp_vcols[g])
                    sp_vcols[g].append((j, w, s))
    NVSP = max(len(v) for v in sp_vcols) if any(sp_vcols) else 0
    NVSP_TOT = NG * max(NVSP, 1)

    # aux slot: rank within (core, w, s) among aux edges
    skey = (core * NW + e_w) * 2 + e_s
    so = np.argsort(skey[aux], kind="stable")
    sko = skey[aux][so]
    sstarts = np.searchsorted(sko, np.arange(NCORES * NW * 2))
    srank = np.empty(aux.sum(), np.int64)
    srank[so] = np.arange(aux.sum()) - sstarts[sko]

    # --- build per-core upload arrays ---
    x16 = np.zeros((NPAD, D), np.float16)
    x16[:N_NODES] = np.asarray(x, np.float32).astype(np.float16)
    x8 = x16.astype(NP_F8)
    res16 = (x16.astype(np.float32) - x8.astype(np.float32)).astype(np.float16)
    wpr = wpr_f.astype(np.float16)

    xexp8 = np.zeros((NCORES, WIN, NG * TILES_MAIN, D), NP_F8)
    xexpa = np.zeros((NCORES, WIN, NG * SPA, D), np.float16)
    wpr_t = np.zeros((NCORES, WIN, NG * TILES_MAIN), np.float16)
    drb_t = np.full((NCORES, WIN, NG * TILES_MAIN), -1.0, np.float16)
    wpr_sp = np.zeros((NCORES, WIN, NVSP_TOT), np.float16)
    dra_sp = np.full((NCORES, WIN, NVSP_TOT), -1.0, np.float16)
    drb_sp = np.full((NCORES, WIN, NVSP_TOT), -1.0, np.float16)

    # main edges (all non-overflow, in fp8)
    em = main
    jm_glob = e_g[em] * TILES_MAIN + bin_tile_in_call[em]
    p_m = rank[em]
    xexp8[core[em], p_m, jm_glob] = x8[src[em]]
    wpr_t[core[em], p_m, jm_glob] = wpr[em]
    drb_t[core[em], p_m, jm_glob] = e_drb[em].astype(np.float16)

    # aux edges: overflow carry full x16, residual-corrections carry x16-x8
    es_idx = np.where(aux)[0]
    is_ovf = spill[es_idx]
    sw, ss, sg, sc = e_w[es_idx], e_s[es_idx], e_g[es_idx], core[es_idx]
    soff = sp_base[sw, ss] + srank
    sj = soff // WIN
    sp_p = soff % WIN
    if len(es_idx):
        vids = np.array([sp_vcol_id[(g_, j_, w_, s_)]
                         for g_, j_, w_, s_ in zip(sg, sj, sw, ss)], np.int64)
        v_glob = sg * max(NVSP, 1) + vids
        j_sp_glob = sg * SPA + sj
        vals = np.where(is_ovf[:, None], x16[src[es_idx]], res16[src[es_idx]])
        xexpa[sc, sp_p, j_sp_glob] = vals.astype(np.float16)
        wpr_sp[sc, sp_p, v_glob] = wpr[es_idx]
        pos_sp = node_pos[dst[es_idx]]
        dra_sp[sc, sp_p, v_glob] = ((pos_sp % 64) // 8).astype(np.float16)
        drb_sp[sc, sp_p, v_glob] = (pos_sp % 8).astype(np.float16)

    # xw: dense x rows per (pos, w) for the +x residual
    rows = np.zeros((NCORES, WIN, NW), np.int64)
    for c in range(NCORES):
        nodes = np.where(node_core == c)[0]
        rows[c, node_pos[nodes], node_w[nodes]] = nodes
    xw = x16[rows]  # [NCORES, 128, NW, D]

    return dict(SPA=SPA, NVSP=max(NVSP, 1),
                NVSP_TOT=NVSP_TOT, sp_vcols=sp_vcols, rows=rows,
                xexp8=xexp8, xexpa=xexpa, wpr_t=wpr_t, drb_t=drb_t,
                wpr_sp=wpr_sp, dra_sp=dra_sp, drb_sp=drb_sp, xw=xw,
                aux_count=int(aux.sum()))


def _build_nc(prep):
    SPA = prep["SPA"]
    NVSP = prep["NVSP"]
    sp_vcols = prep["sp_vcols"]

    NTM = NG * TILES_MAIN
    NVT = prep["NVSP_TOT"]

    nc = bacc.Bacc(num_devices=NCORES)
    xexp8_t = nc.dram_tensor("xexp8", [WIN, NTM * D], F8,
                             kind="ExternalInput")
    xexpa_t = nc.dram_tensor("xexpa", [WIN, NG * SPA * D], F16,
                             kind="ExternalInput")
    mtab_t = nc.dram_tensor("mtab", [WIN, 2 * NTM], F16, kind="ExternalInput")
    stab_t = nc.dram_tensor("stab", [WIN, 3 * NVT], F16, kind="ExternalInput")
    xwb_t = nc.dram_tensor("xwb", [WIN, NW * D + D], F16, kind="ExternalInput")
    out_t = nc.dram_tensor("out", [WIN, NW, D], F16, kind="ExternalOutput")

    with tile.TileContext(nc) as tc:
        from contextlib import ExitStack

        with ExitStack() as ctx:
            const = ctx.enter_context(tc.tile_pool(name="const", bufs=1))
            gp = ctx.enter_context(tc.tile_pool(name="gp", bufs=2))
            gpa = ctx.enter_context(tc.tile_pool(name="gpa", bufs=2))
            ohp = ctx.enter_context(tc.tile_pool(name="ohp", bufs=2))
            osp = ctx.enter_context(tc.tile_pool(name="osp", bufs=2))
            abp = ctx.enter_context(tc.tile_pool(name="abp", bufs=2))
            aggp = ctx.enter_context(tc.tile_pool(name="aggp", bufs=3))
            psw = ctx.enter_context(tc.tile_pool(name="psw", bufs=1, space="PSUM"))
            psr = ctx.enter_context(tc.tile_pool(name="psr", bufs=1, space="PSUM"))

            # G for call 0 first: it is the longest transfer on the
            # critical path, so it must hit the DMA engines before the
            # constant tables.
            G0 = gp.tile([WIN, TILES_MAIN, D], F8, tag="g0")
            nc.sync.dma_start(out=G0[:, :, :], in_=xexp8_t[:, :TILES_MAIN * D])
            GA0 = gpa.tile([WIN, SPA, D], F16, tag="a0")
            nc.sync.dma_start(out=GA0[:, :, :], in_=xexpa_t[:, :SPA * D])

            mtab = const.tile([WIN, 2 * NTM], F16)
            nc.sync.dma_start(out=mtab[:, :], in_=mtab_t[:, :])
            wprm = mtab[:, :NTM]
            drbm = mtab[:, NTM:]
            xwb = const.tile([WIN, NW * D + D], F16)
            nc.sync.dma_start(out=xwb[:, :], in_=xwb_t[:, :])
            wbt = xwb[:D + 1, NW * D:]
            stab = const.tile([WIN, 3 * NVT], F16)
            nc.sync.dma_start(out=stab[:, :], in_=stab_t[:, :])
            wprs = stab[:, :NVT]
            dras = stab[:, NVT:2 * NVT]
            drbs = stab[:, 2 * NVT:]

            ident16 = const.tile([WIN, WIN], F16)
            make_identity(nc, ident16[:, :])
            iota8 = const.tile([WIN, 8], F16)
            nc.gpsimd.iota(iota8[:, :], pattern=[[1, 8]], base=0,
                           channel_multiplier=0,
                           allow_small_or_imprecise_dtypes=True)

            outr = const.tile([WIN, NW, D], F16)

            # pre-set the ones row of the agg buffers (one per window-in-group
            # so a whole group's updates can be in flight at once)
            aggs = []
            for k in range(GW):
                agg = aggp.tile([D + 1, WIN], F16, tag=f"agg{k}")
                nc.vector.memset(agg[D:D + 1, :], 1.0)
                aggs.append(agg)

            for g in range(NG):
                if g == 0:
                    G, GA = G0, GA0
                else:
                    G = gp.tile([WIN, TILES_MAIN, D], F8, tag=f"g{g % 2}")
                    nc.sync.dma_start(
                        out=G[:, :, :],
                        in_=xexp8_t[:, g * TILES_MAIN * D:(g + 1) * TILES_MAIN * D])
                    GA = gpa.tile([WIN, SPA, D], F16, tag=f"a{g % 2}")
                    nc.sync.dma_start(
                        out=GA[:, :, :],
                        in_=xexpa_t[:, g * SPA * D:(g + 1) * SPA * D])

                # 8-wide one-hot for the 112 main tiles of this call
                OH8 = ohp.tile([WIN, TILES_MAIN, 8], F16, tag=f"oh{g % 2}")
                nc.vector.tensor_tensor(
                    out=OH8[:, :, :],
                    in0=iota8[:, None, :].to_broadcast([WIN, TILES_MAIN, 8]),
                    in1=drbm[:, g * TILES_MAIN:(g + 1) * TILES_MAIN, None]
                        .to_broadcast([WIN, TILES_MAIN, 8]),
                    op=mybir.AluOpType.is_equal,
                )
                nc.vector.tensor_tensor(
                    out=OH8[:, :, :],
                    in0=OH8[:, :, :],
                    in1=wprm[:, g * TILES_MAIN:(g + 1) * TILES_MAIN, None]
                        .to_broadcast([WIN, TILES_MAIN, 8]),
                    op=mybir.AluOpType.mult,
                )

                # 64-wide one-hot for spill vcols of this call
                nv = len(sp_vcols[g])
                OHS = None
                if nv:
                    eng = nc.vector
                    v0 = g * NVSP
                    ohA = abp.tile([WIN, NVSP, 8], F16, tag=f"a{g % 2}")
                    eng.tensor_tensor(
                        out=ohA[:, :nv, :],
                        in0=iota8[:, None, :].to_broadcast([WIN, nv, 8]),
                        in1=dras[:, v0:v0 + nv, None].to_broadcast([WIN, nv, 8]),
                        op=mybir.AluOpType.is_equal,
                    )
                    eng.tensor_tensor(
                        out=ohA[:, :nv, :],
                        in0=ohA[:, :nv, :],
                        in1=wprs[:, v0:v0 + nv, None].to_broadcast([WIN, nv, 8]),
                        op=mybir.AluOpType.mult,
                    )
                    ohB = abp.tile([WIN, NVSP, 8], F16, tag=f"b{g % 2}")
                    eng.tensor_tensor(
                        out=ohB[:, :nv, :],
                        in0=iota8[:, None, :].to_broadcast([WIN, nv, 8]),
                        in1=drbs[:, v0:v0 + nv, None].to_broadcast([WIN, nv, 8]),
                        op=mybir.AluOpType.is_equal,
                    )
                    OHS = osp.tile([WIN, NVSP, 8, 8], F16, tag=f"o{g % 2}")
                    eng.tensor_tensor(
                        out=OHS[:, :nv, :, :],
                        in0=ohA[:, :nv, :, None].to_broadcast([WIN, nv, 8, 8]),
                        in1=ohB[:, :nv, None, :].to_broadcast([WIN, nv, 8, 8]),
                        op=mybir.AluOpType.mult,
                    )

                pss = []
                for wl in range(GW):
                    w = g * GW + wl
                    myspill = [(k, j, s) for k, (j, w_, s) in enumerate(sp_vcols[g])
                               if w_ == w]
                    ntot = NBIN_W + len(myspill)
                    ps = psw.tile([D, WIN], F32, tag=f"ps{wl % 6}")
                    nc.tensor.matmul(out=ps[:, :], lhsT=xwb[:, w * D:(w + 1) * D],
                                     rhs=ident16[:, :], start=True, stop=False,
                                     skip_group_check=True)
                    done = 0
                    for s in range(2):
                        for A in range(8):
                            jm = wl * NBIN_W + s * 8 + A
                            done += 1
                            nc.tensor.matmul(
                                out=ps[:, s * 64 + A * 8: s * 64 + A * 8 + 8],
                                lhsT=G[:, jm, :],
                                rhs=OH8[:, jm, :],
                                start=False, stop=(done == ntot),
                                skip_group_check=True,
                            )
                    for (k, j, s) in myspill:
                        done += 1
                        nc.tensor.matmul(
                            out=ps[:, s * 64:(s + 1) * 64],
                            lhsT=GA[:, j, :],
                            rhs=OHS[:, k, :, :],
                            start=False, stop=(done == ntot),
                            skip_group_check=True,
                        )
                    pss.append(ps)
                    nc.scalar.copy(out=aggs[wl][:D, :], in_=ps[:, :])
                for wl in range(GW):
                    w = g * GW + wl
                    rp = psr.tile([WIN, D], F32, tag=f"rp{wl % 2}")
                    nc.tensor.matmul(out=rp[:, :], lhsT=aggs[wl][:, :],
                                     rhs=wbt[:, :], start=True, stop=True,
                                     skip_group_check=True)
                    nc.scalar.copy(out=outr[:, w, :], in_=rp[:, :])
                nc.sync.dma_start(
                    out=out_t[:, g * GW:(g + 1) * GW, :],
                    in_=outr[:, g * GW:(g + 1) * GW, :])

    nc.compile()
    return nc


def kernel(x, edge_index, edge_weight, pagerank, W, b):
    x = np.asarray(x, np.float32)
    pr = np.asarray(pagerank, np.float32)
    W = np.asarray(W, np.float32)
    b = np.asarray(b, np.float32)

    prep = _host_prep(x, edge_index, edge_weight, pr)
    nc = _build_nc(prep)

    wbt = np.zeros((WIN, D), np.float16)
    wbt[:D] = W.T.astype(np.float16)
    wbt[D] = b.astype(np.float16)

    in_maps = []
    for c in range(NCORES):
        mtab = np.concatenate([prep["wpr_t"][c], prep["drb_t"][c]], axis=1)
        stab = np.concatenate([prep["wpr_sp"][c], prep["dra_sp"][c],
                               prep["drb_sp"][c]], axis=1)
        xwb = np.concatenate([prep["xw"][c].reshape(WIN, NW * D), wbt], axis=1)
        in_maps.append({
            "xexp8": prep["xexp8"][c].reshape(WIN, NG * TILES_MAIN * D),
            "xexpa": prep["xexpa"][c].reshape(WIN, NG * prep["SPA"] * D),
            "mtab": np.ascontiguousarray(mtab),
            "stab": np.ascontiguousarray(stab),
            "xwb": np.ascontiguousarray(xwb),
        })

    import time

    t0 = time.time()
    res = run_bass_kernel_spmd(nc, in_maps, core_ids=list(range(NCORES)))
    _LAST.update(nc=nc, run_wall_s=time.time() - t0)

    rows = prep["rows"]
    out = np.zeros((NPAD, D), np.float32)
    for c in range(NCORES):
        o = res.results[c]["out"].astype(np.float32)  # [128, NW, 96]
        out[rows[c]] = o
    return out[:N_NODES]
